# revision 11
# baseline (speedup 1.0000x reference)
"""Trainium2 kernel for the DepthTracker correlation pyramid.

Math: for each level l, frame t, track n, the reference bilinearly samples a
7x7 grid of points around coords[t,n] from fmaps_l (128 channels) and
correlates each sample with the 49 track features -> out (L,B,T,N,7,7,7,7).

Decomposition (device does the 10-GFLOP correlation einsum, host does the
0.6-GFLOP bilinear gather/blend + layout):
  out[l,t,n,h,w,pq] = G[l,n,pq,(t,h,w)]
  G[l,n,pq,tuv]     = sum_c track[c,(l,n,pq)] * patch[l,c,(n,tuv)]
where patch is the host-blended 7x7 sample grid (border clamping folded in).

Quantization: patches ship as fp8 e3m4 (4 mantissa bits), scaled per (l,n)
so the max |value| hits the e3m4 max (15.5); the inverse scale is folded
into the f16 track operand. Measured end-to-end rel err 1.45e-2 (gate 2e-2).
This halves the dominant DMA stream (25.7 -> 12.85 MB/core).

Device matmul orientation: stationary = fp8 patch chunk [C=128, 112],
moving = f16 track [C=128, 49] -> PSUM [112, 49] at 64-f32 col stride in a
single bank tile [112, 448]. 7 chunks cover tuv=784. This (a) uses 112/128
PE columns (vs 49/128 track-stationary), (b) makes the G store tiles
112-real-partition (no pad rows; 14/16 SDMA engines), (c) shrinks the
PSUM->SBUF copies to [112, 343] each.

Per-core DMA: 12.85 MB patches (fp8) + 1.6 MB track (f16) in, 9.83 MB G
(f16) out = 24.3 MB vs the f16 baseline's 40.1 MB.
"""

import numpy as np

R = 3
K7 = 7
LEV = 4
B, T, C, N = 1, 16, 128, 256
H, W = 96, 128
NCORES = 8
NS = N // NCORES          # 32 tracks per core
TUV = T * K7 * K7         # 784 samples per track
PQ = K7 * K7              # 49
CHK = 112                 # tuv chunk per matmul (stationary free dim)
NCH = TUV // CHK          # 7 chunks
PSTRIDE = 64              # PSUM col stride per chunk (f32), 256B-aligned

PATCH_DT = 'f8e3'         # 'f8e3' | 'f16'  (fallback if mixed-dtype MM fails)
TRACE = False             # set True to capture an NTFF profile (test.py only)
LAST_RESULT = {}          # phase timings + profile info for test.py

# load batches per level: small first batch halves ramp, small last batches
# shrink the store tail
BATCHES = {0: (8, 8, 8, 8), 1: (16, 16), 2: (16, 16), 3: (16, 8, 4, 2, 2)}

_BASS_CACHE = {}


def _np_patch_dtype():
    if PATCH_DT == 'f16':
        return np.float16
    import ml_dtypes
    return np.dtype(ml_dtypes.float8_e3m4)


def _build_bass():
    key = (PATCH_DT,)
    if key in _BASS_CACHE:
        return _BASS_CACHE[key]
    import concourse.bacc as bacc
    import concourse.mybir as mybir
    from concourse import tile

    pdt = mybir.dt.float8e3 if PATCH_DT == 'f8e3' else mybir.dt.float16
    f16 = mybir.dt.float16
    f32 = mybir.dt.float32

    nc = bacc.Bacc("TRN2", target_bir_lowering=False, debug=False)
    # c-major, each partition's (n, tuv) block contiguous per level.
    # +16 pad cols: every matmul loads a full 128-col stationary (7th chunk
    # reads 16 cols past the track) so Fast Weight Load stays enabled; the
    # garbage lands in PSUM rows 112:128 which are never copied out.
    patches = nc.dram_tensor("patches", (LEV, C, NS * TUV + 16), pdt,
                             kind="ExternalInput")
    trackT = nc.dram_tensor("trackT", (C, LEV * NS * PQ), f16,
                            kind="ExternalInput")
    # G^T per level: [p(=tuv%112... really tuv = c*112+p), n*343 + c*49 + q]
    # per-partition contiguous cols -> large store packets, no pad rows
    gout = nc.dram_tensor("gout", (LEV, CHK, NS * NCH * PQ), f16,
                          kind="ExternalOutput")

    # load order: level-0 track is split so the first patch batch starts
    # almost immediately; each level's track slice precedes its batches
    batches = [(l, i, sum(BATCHES[l][:i]), nb)
               for l in range(LEV) for i, nb in enumerate(BATCHES[l])]

    with tile.TileContext(nc) as tc:
        with (
            tc.tile_pool(name="track", bufs=1) as track_pool,
            tc.tile_pool(name="patch", bufs=4) as patch_pool,
            tc.tile_pool(name="out", bufs=4) as out_pool,
            tc.tile_pool(name="psum", bufs=8, space="PSUM") as psum_pool,
        ):
            tr = track_pool.tile([C, LEV * NS * PQ], f16)

            loads = []           # deferred load issuers, run AHEAD batches early
            patch_idx = {}       # bi -> index in `loads` of its patch load
            patch_tiles = {}

            def load_track(l, n0, n1):
                def go():
                    a = (l * NS + n0) * PQ
                    b = (l * NS + n1) * PQ
                    nc.scalar.dma_start(tr[:, a:b], trackT[:, a:b])
                return go

            def load_patch(bi, l, off, nb):
                # alternate trigger queues so SDMA engines always have
                # packets queued from two independent rings
                eng = nc.sync if bi % 2 == 0 else nc.scalar
                def go():
                    pt = patch_pool.tile([C, nb * TUV + 16], pdt, tag="pt",
                                         name="pt")
                    patch_tiles[bi] = pt
                    a, b = off * TUV, (off + nb) * TUV + 16
                    if bi == 0:
                        m = (a + b) // 2
                        nc.sync.dma_start(pt[:, :m - a], patches[l, :, a:m])
                        nc.scalar.dma_start(pt[:, m - a:], patches[l, :, m:b])
                    else:
                        eng.dma_start(pt[:], patches[l, :, a:b])
                return go

            for bi, (l, i, off, nb) in enumerate(batches):
                if i == 0:
                    if l == 0:
                        loads.append(load_track(0, 0, BATCHES[0][0]))
                    else:
                        loads.append(load_track(l, 0, NS))
                loads.append(load_patch(bi, l, off, nb))
                patch_idx[bi] = len(loads) - 1
                if l == 0 and i == 0:
                    loads.append(load_track(0, BATCHES[0][0], NS))

            AHEAD = 3            # batches of load prefetch
            li = 0
            for bi, (l, i, off, nb) in enumerate(batches):
                need = patch_idx[min(bi + AHEAD, len(batches) - 1)]
                while li <= need:
                    loads[li]()
                    li += 1
                pt = patch_tiles[bi]
                ot = out_pool.tile([CHK, nb * NCH * PQ], f16, tag="ot",
                                   name="ot")
                for g in range(nb):
                    n = off + g
                    k = (l * NS + n) * PQ
                    ps = psum_pool.tile([128, NCH * PSTRIDE], f32, tag="ps",
                                        name="ps")
                    for c in range(NCH):
                        nc.tensor.matmul(
                            ps[:, c * PSTRIDE:c * PSTRIDE + PQ],
                            pt[:, g * TUV + c * CHK:g * TUV + c * CHK + 128],
                            tr[:, k:k + PQ],
                            start=True, stop=True)
                    src = ps[0:CHK, :].rearrange("p (c x) -> p c x",
                                                 c=NCH)[:, :, 0:PQ]
                    dst = ot[:, g * NCH * PQ:(g + 1) * NCH * PQ].rearrange(
                        "p (c x) -> p c x", c=NCH)
                    if g % 2 == 0:
                        nc.vector.tensor_copy(dst, src)
                    else:
                        nc.scalar.copy(dst, src)
                # stores ride the (otherwise idle) GpSimd trigger queue so
                # they never head-of-line-block patch loads on the sync ring
                nc.gpsimd.dma_start(
                    gout[l, :, off * NCH * PQ:(off + nb) * NCH * PQ], ot[:])
            while li < len(loads):
                loads[li]()
                li += 1
    nc.compile()
    _BASS_CACHE[key] = nc
    return nc


def _blend_mats(xy, dim):
    """xy: (T,N) fp32 coords at this level's scale. Returns (origin (T,N)
    int32, S (T,N,7,8) fp32) with reference clamping semantics folded in."""
    d = np.arange(-R, R + 1, dtype=np.float32)
    q = xy[..., None] + d
    qc = np.clip(q, 0.0, dim - 1.0)
    x0 = np.floor(qc)
    w = (qc - x0).astype(np.float32)
    x0i = x0.astype(np.int32)
    x1i = np.minimum(x0i + 1, dim - 1)
    org = np.clip(np.floor(xy).astype(np.int32) - R, 0, dim - 8)
    v0 = x0i - org[..., None]
    v1 = x1i - org[..., None]
    eye = np.eye(8, dtype=np.float32)
    S = eye[v0] * (1.0 - w)[..., None] + eye[v1] * w[..., None]
    return org, S


def kernel(fmaps0, fmaps1, fmaps2, fmaps3, track0, track1, track2, track3,
           coords):
    import time as _time
    _t0 = _time.time()
    fmaps = [fmaps0, fmaps1, fmaps2, fmaps3]
    tracks = [track0, track1, track2, track3]
    pdt_np = _np_patch_dtype()
    coords2 = np.asarray(coords, np.float32)[0]        # (T,N,2)

    # ---- host: blend matrices + patch gather + fp8 quantization -------------
    FMAX = 15.5 if PATCH_DT == 'f8e3' else 1.0
    patches_all = np.empty((LEV, C, N, T, K7, K7), pdt_np)
    scale = np.empty((LEV, N), np.float32)
    for l in range(LEV):
        Hl, Wl = H >> l, W >> l
        sc = np.float32(2.0 ** l)
        x = (coords2[..., 0] / sc).astype(np.float32)
        y = (coords2[..., 1] / sc).astype(np.float32)
        cx, Sx = _blend_mats(x, Wl)
        cy, Sy = _blend_mats(y, Hl)
        fm = np.asarray(fmaps[l], np.float32)[0]       # (T,C,Hl,Wl)
        iy = cy[..., None] + np.arange(8)              # (T,N,8)
        ix = cx[..., None] + np.arange(8)
        t_idx = np.arange(T)[:, None, None, None]
        # fancy indexing -> (T,N,8,8,C) over (u=y-row, v=x-col)
        p = fm[t_idx, :, iy[:, :, :, None], ix[:, :, None, :]]
        # x-blend: (T,N,1,7,8) @ (T,N,8,8,C) -> (T,N,8,7,C)
        px = np.matmul(Sx[:, :, None, :, :], p)
        # y-blend: (T,N,7,8) @ (T,N,8,7*C) -> (T,N,7,7,C)
        py = np.matmul(Sy, px.reshape(T, N, 8, K7 * C))
        py = py.reshape(T, N, K7, K7, C)               # (T,N,h,w,C)
        if PATCH_DT == 'f8e3':
            pmax = np.abs(py).max(axis=(0, 2, 3, 4))   # per track n
            scale[l] = FMAX / pmax
            py = py * scale[l][None, :, None, None, None]
        else:
            scale[l] = 1.0
        patches_all[l] = py.transpose(4, 1, 0, 2, 3)   # (C,N,T,7,7)

    trackT_all = np.empty((C, LEV, N, PQ), np.float16)
    for l in range(LEV):
        # track_l: (1,49,N,C) -> (C,N,PQ), divided by the patch scale
        tl = np.asarray(tracks[l], np.float32)[0].transpose(2, 1, 0)
        trackT_all[:, l] = tl / scale[l][None, :, None]

    # ---- device: G^T = patches^T @ track, 32 tracks per core ----------------
    nc = _build_bass()
    from concourse import bass_utils
    in_maps = []
    for k in range(NCORES):
        sl = slice(k * NS, (k + 1) * NS)
        pk = np.zeros((LEV, C, NS * TUV + 16), pdt_np)
        pk[:, :, :NS * TUV] = patches_all[:, :, sl].reshape(LEV, C, NS * TUV)
        in_maps.append({
            "patches": pk,
            "trackT": np.ascontiguousarray(
                trackT_all[:, :, sl].reshape(C, LEV * NS * PQ)),
        })
    _t1 = _time.time()
    res = bass_utils.run_bass_kernel_spmd(
        nc, in_maps, core_ids=list(range(NCORES)), trace=TRACE)
    _t2 = _time.time()
    LAST_RESULT.update(
        host_pre_s=_t1 - _t0, spmd_s=_t2 - _t1,
        exec_time_ns=res.exec_time_ns, profile_json=res.profile_json)
    # per core: gout (LEV, 112, NS*7*49): [l, p, n*343 + c*49 + q] with
    # tuv = c*112 + p. Reassemble G[l, n, q, tuv].
    G = np.empty((LEV, N, PQ, TUV), np.float32)
    for kc, r in enumerate(res.results):
        g = r["gout"].reshape(LEV, CHK, NS, NCH, PQ)
        # -> (LEV, NS, q, c, p)
        G[:, kc * NS:(kc + 1) * NS] = g.transpose(0, 2, 4, 3, 1).reshape(
            LEV, NS, PQ, TUV)
    G = G.reshape(LEV, N, PQ, T, K7, K7)       # [l,n,q,t,h,w]

    # ---- host: final layout only --------------------------------------------
    out = np.ascontiguousarray(
        G.transpose(0, 3, 1, 5, 4, 2), dtype=np.float32).reshape(
        LEV, B, T, N, K7, K7, K7, K7)
    LAST_RESULT['host_post_s'] = _time.time() - _t2
    return out


# revision 12
# speedup vs baseline: 1.0156x; 1.0156x over previous
"""Trainium2 kernel for the DepthTracker correlation pyramid.

Math: for each level l, frame t, track n, the reference bilinearly samples a
7x7 grid of points around coords[t,n] from fmaps_l (128 channels) and
correlates each sample with the 49 track features -> out (L,B,T,N,7,7,7,7).

Decomposition (device does the 10-GFLOP correlation einsum, host does the
0.6-GFLOP bilinear gather/blend + layout):
  out[l,t,n,h,w,pq] = G[l,n,pq,(t,h,w)]
  G[l,n,pq,tuv]     = sum_c track[c,(l,n,pq)] * patch[l,c,(n,tuv)]
where patch is the host-blended 7x7 sample grid (border clamping folded in).

Quantization: patches ship as fp8 e3m4 (4 mantissa bits), scaled per (l,n)
so the max |value| hits the e3m4 max (15.5); the inverse scale is folded
into the f16 track operand. Measured end-to-end rel err 1.45e-2 (gate 2e-2).
This halves the dominant DMA stream (25.7 -> 12.85 MB/core).

Device matmul orientation: stationary = fp8 patch chunk [C=128, 112],
moving = f16 track [C=128, 49] -> PSUM [112, 49] at 64-f32 col stride in a
single bank tile [112, 448]. 7 chunks cover tuv=784. This (a) uses 112/128
PE columns (vs 49/128 track-stationary), (b) makes the G store tiles
112-real-partition (no pad rows; 14/16 SDMA engines), (c) shrinks the
PSUM->SBUF copies to [112, 343] each.

Per-core DMA: 12.85 MB patches (fp8) + 1.6 MB track (f16) in, 9.83 MB G
(f16) out = 24.3 MB vs the f16 baseline's 40.1 MB.
"""

import numpy as np

R = 3
K7 = 7
LEV = 4
B, T, C, N = 1, 16, 128, 256
H, W = 96, 128
NCORES = 8
NS = N // NCORES          # 32 tracks per core
TUV = T * K7 * K7         # 784 samples per track
PQ = K7 * K7              # 49
CHK = 112                 # tuv chunk per matmul (stationary free dim)
NCH = TUV // CHK          # 7 chunks
PSTRIDE = 64              # PSUM col stride per chunk (f32), 256B-aligned

PATCH_DT = 'f8e3'         # 'f8e3' | 'f16'  (fallback if mixed-dtype MM fails)
TRACE = False             # set True to capture an NTFF profile (test.py only)
LAST_RESULT = {}          # phase timings + profile info for test.py

# load batches per level: small first batch halves ramp, small last batches
# shrink the store tail
BATCHES = {0: (4, 4, 8, 8, 8), 1: (16, 16), 2: (16, 16), 3: (16, 8, 4, 2, 2)}

_BASS_CACHE = {}


def _np_patch_dtype():
    if PATCH_DT == 'f16':
        return np.float16
    import ml_dtypes
    return np.dtype(ml_dtypes.float8_e3m4)


def _build_bass():
    key = (PATCH_DT,)
    if key in _BASS_CACHE:
        return _BASS_CACHE[key]
    import concourse.bacc as bacc
    import concourse.mybir as mybir
    from concourse import tile

    pdt = mybir.dt.float8e3 if PATCH_DT == 'f8e3' else mybir.dt.float16
    f16 = mybir.dt.float16
    f32 = mybir.dt.float32

    nc = bacc.Bacc("TRN2", target_bir_lowering=False, debug=False)
    # c-major, each partition's (n, tuv) block contiguous per level.
    # +16 pad cols: every matmul loads a full 128-col stationary (7th chunk
    # reads 16 cols past the track) so Fast Weight Load stays enabled; the
    # garbage lands in PSUM rows 112:128 which are never copied out.
    patches = nc.dram_tensor("patches", (LEV, C, NS * TUV + 16), pdt,
                             kind="ExternalInput")
    trackT = nc.dram_tensor("trackT", (C, LEV * NS * PQ), f16,
                            kind="ExternalInput")
    # G^T per level: [p(=tuv%112... really tuv = c*112+p), n*343 + c*49 + q]
    # per-partition contiguous cols -> large store packets, no pad rows
    gout = nc.dram_tensor("gout", (LEV, CHK, NS * NCH * PQ), f16,
                          kind="ExternalOutput")

    # load order: level-0 track is split so the first patch batch starts
    # almost immediately; each level's track slice precedes its batches
    batches = [(l, i, sum(BATCHES[l][:i]), nb)
               for l in range(LEV) for i, nb in enumerate(BATCHES[l])]

    with tile.TileContext(nc) as tc:
        with (
            tc.tile_pool(name="track", bufs=1) as track_pool,
            tc.tile_pool(name="patch", bufs=5) as patch_pool,
            tc.tile_pool(name="out", bufs=4) as out_pool,
            tc.tile_pool(name="psum", bufs=8, space="PSUM") as psum_pool,
        ):
            tr = track_pool.tile([C, LEV * NS * PQ], f16)

            loads = []           # deferred load issuers, run AHEAD batches early
            patch_idx = {}       # bi -> index in `loads` of its patch load
            patch_tiles = {}

            def load_track(l, n0, n1):
                def go():
                    a = (l * NS + n0) * PQ
                    b = (l * NS + n1) * PQ
                    nc.scalar.dma_start(tr[:, a:b], trackT[:, a:b])
                return go

            def load_patch(bi, l, off, nb):
                # alternate trigger queues so SDMA engines always have
                # packets queued from two independent rings
                eng = nc.sync if bi % 2 == 0 else nc.scalar
                def go():
                    pt = patch_pool.tile([C, nb * TUV + 16], pdt, tag="pt",
                                         name="pt")
                    patch_tiles[bi] = pt
                    a, b = off * TUV, (off + nb) * TUV + 16
                    if bi == 0:
                        m = (a + b) // 2
                        nc.sync.dma_start(pt[:, :m - a], patches[l, :, a:m])
                        nc.scalar.dma_start(pt[:, m - a:], patches[l, :, m:b])
                    else:
                        eng.dma_start(pt[:], patches[l, :, a:b])
                return go

            for bi, (l, i, off, nb) in enumerate(batches):
                if i == 0:
                    if l == 0:
                        loads.append(load_track(0, 0, BATCHES[0][0]))
                    else:
                        loads.append(load_track(l, 0, NS))
                loads.append(load_patch(bi, l, off, nb))
                patch_idx[bi] = len(loads) - 1
                if l == 0 and i == 0:
                    loads.append(load_track(0, BATCHES[0][0], NS))

            AHEAD = 4            # batches of load prefetch
            li = 0
            for bi, (l, i, off, nb) in enumerate(batches):
                need = patch_idx[min(bi + AHEAD, len(batches) - 1)]
                while li <= need:
                    loads[li]()
                    li += 1
                pt = patch_tiles[bi]
                ot = out_pool.tile([CHK, nb * NCH * PQ], f16, tag="ot",
                                   name="ot")
                for g in range(nb):
                    n = off + g
                    k = (l * NS + n) * PQ
                    ps = psum_pool.tile([128, NCH * PSTRIDE], f32, tag="ps",
                                        name="ps")
                    for c in range(NCH):
                        nc.tensor.matmul(
                            ps[:, c * PSTRIDE:c * PSTRIDE + PQ],
                            pt[:, g * TUV + c * CHK:g * TUV + c * CHK + 128],
                            tr[:, k:k + PQ],
                            start=True, stop=True)
                    src = ps[0:CHK, :].rearrange("p (c x) -> p c x",
                                                 c=NCH)[:, :, 0:PQ]
                    dst = ot[:, g * NCH * PQ:(g + 1) * NCH * PQ].rearrange(
                        "p (c x) -> p c x", c=NCH)
                    if g % 2 == 0:
                        nc.vector.tensor_copy(dst, src)
                    else:
                        nc.scalar.copy(dst, src)
                # stores ride the (otherwise idle) GpSimd trigger queue so
                # they never head-of-line-block patch loads on the sync ring
                nc.gpsimd.dma_start(
                    gout[l, :, off * NCH * PQ:(off + nb) * NCH * PQ], ot[:])
            while li < len(loads):
                loads[li]()
                li += 1
    nc.compile()
    _BASS_CACHE[key] = nc
    return nc


def _blend_mats(xy, dim):
    """xy: (T,N) fp32 coords at this level's scale. Returns (origin (T,N)
    int32, S (T,N,7,8) fp32) with reference clamping semantics folded in."""
    d = np.arange(-R, R + 1, dtype=np.float32)
    q = xy[..., None] + d
    qc = np.clip(q, 0.0, dim - 1.0)
    x0 = np.floor(qc)
    w = (qc - x0).astype(np.float32)
    x0i = x0.astype(np.int32)
    x1i = np.minimum(x0i + 1, dim - 1)
    org = np.clip(np.floor(xy).astype(np.int32) - R, 0, dim - 8)
    v0 = x0i - org[..., None]
    v1 = x1i - org[..., None]
    eye = np.eye(8, dtype=np.float32)
    S = eye[v0] * (1.0 - w)[..., None] + eye[v1] * w[..., None]
    return org, S


def kernel(fmaps0, fmaps1, fmaps2, fmaps3, track0, track1, track2, track3,
           coords):
    import time as _time
    _t0 = _time.time()
    fmaps = [fmaps0, fmaps1, fmaps2, fmaps3]
    tracks = [track0, track1, track2, track3]
    pdt_np = _np_patch_dtype()
    coords2 = np.asarray(coords, np.float32)[0]        # (T,N,2)

    # ---- host: blend matrices + patch gather + fp8 quantization -------------
    FMAX = 15.5 if PATCH_DT == 'f8e3' else 1.0
    patches_all = np.empty((LEV, C, N, T, K7, K7), pdt_np)
    scale = np.empty((LEV, N), np.float32)
    for l in range(LEV):
        Hl, Wl = H >> l, W >> l
        sc = np.float32(2.0 ** l)
        x = (coords2[..., 0] / sc).astype(np.float32)
        y = (coords2[..., 1] / sc).astype(np.float32)
        cx, Sx = _blend_mats(x, Wl)
        cy, Sy = _blend_mats(y, Hl)
        fm = np.asarray(fmaps[l], np.float32)[0]       # (T,C,Hl,Wl)
        iy = cy[..., None] + np.arange(8)              # (T,N,8)
        ix = cx[..., None] + np.arange(8)
        t_idx = np.arange(T)[:, None, None, None]
        # fancy indexing -> (T,N,8,8,C) over (u=y-row, v=x-col)
        p = fm[t_idx, :, iy[:, :, :, None], ix[:, :, None, :]]
        # x-blend: (T,N,1,7,8) @ (T,N,8,8,C) -> (T,N,8,7,C)
        px = np.matmul(Sx[:, :, None, :, :], p)
        # y-blend: (T,N,7,8) @ (T,N,8,7*C) -> (T,N,7,7,C)
        py = np.matmul(Sy, px.reshape(T, N, 8, K7 * C))
        py = py.reshape(T, N, K7, K7, C)               # (T,N,h,w,C)
        if PATCH_DT == 'f8e3':
            pmax = np.abs(py).max(axis=(0, 2, 3, 4))   # per track n
            scale[l] = FMAX / pmax
            py = py * scale[l][None, :, None, None, None]
        else:
            scale[l] = 1.0
        patches_all[l] = py.transpose(4, 1, 0, 2, 3)   # (C,N,T,7,7)

    trackT_all = np.empty((C, LEV, N, PQ), np.float16)
    for l in range(LEV):
        # track_l: (1,49,N,C) -> (C,N,PQ), divided by the patch scale
        tl = np.asarray(tracks[l], np.float32)[0].transpose(2, 1, 0)
        trackT_all[:, l] = tl / scale[l][None, :, None]

    # ---- device: G^T = patches^T @ track, 32 tracks per core ----------------
    nc = _build_bass()
    from concourse import bass_utils
    in_maps = []
    for k in range(NCORES):
        sl = slice(k * NS, (k + 1) * NS)
        pk = np.zeros((LEV, C, NS * TUV + 16), pdt_np)
        pk[:, :, :NS * TUV] = patches_all[:, :, sl].reshape(LEV, C, NS * TUV)
        in_maps.append({
            "patches": pk,
            "trackT": np.ascontiguousarray(
                trackT_all[:, :, sl].reshape(C, LEV * NS * PQ)),
        })
    _t1 = _time.time()
    res = bass_utils.run_bass_kernel_spmd(
        nc, in_maps, core_ids=list(range(NCORES)), trace=TRACE)
    _t2 = _time.time()
    LAST_RESULT.update(
        host_pre_s=_t1 - _t0, spmd_s=_t2 - _t1,
        exec_time_ns=res.exec_time_ns, profile_json=res.profile_json)
    # per core: gout (LEV, 112, NS*7*49): [l, p, n*343 + c*49 + q] with
    # tuv = c*112 + p. Reassemble G[l, n, q, tuv].
    G = np.empty((LEV, N, PQ, TUV), np.float32)
    for kc, r in enumerate(res.results):
        g = r["gout"].reshape(LEV, CHK, NS, NCH, PQ)
        # -> (LEV, NS, q, c, p)
        G[:, kc * NS:(kc + 1) * NS] = g.transpose(0, 2, 4, 3, 1).reshape(
            LEV, NS, PQ, TUV)
    G = G.reshape(LEV, N, PQ, T, K7, K7)       # [l,n,q,t,h,w]

    # ---- host: final layout only --------------------------------------------
    out = np.ascontiguousarray(
        G.transpose(0, 3, 1, 5, 4, 2), dtype=np.float32).reshape(
        LEV, B, T, N, K7, K7, K7, K7)
    LAST_RESULT['host_post_s'] = _time.time() - _t2
    return out
